# revision 1
# baseline (speedup 1.0000x reference)
"""DirGCNConv on 8 Trainium2 NeuronCores via Bass/Tile (v2: scatter-free).

out = (1-a)*(Dout^-1/2 A Din^-1/2 x) @ Wsrc.T + a*(Din^-1/2 A.T Dout^-1/2 x) @ Wdst.T + bias

Per-edge weight separates: w[e] = ao[row[e]] * bi[col[e]], so each direction
is agg[dest] = Sum_{edges} prescaled_x[src], then a per-dest scale + matmul.

v2 strategy (vs v1 gather+scatter-add): edges are sorted by *destination*;
gathered source rows (bf16, dma_gather) are reduced per dest block with
one-hot segment matmuls on the Tensor engine accumulating in PSUM, then
folded into an SBUF-resident [feat x dest] accumulator. This removes all
dma_scatter_add calls — the GpSimd descriptor-generation engine (the
bottleneck) only runs gathers.

SPMD: one program for all 8 cores, so the chunk/matmul schedule is static:
each (window, dest-block) segment is padded to the max token count over
cores. Per-core data (gather indices, one-hot dest columns) differs only in
tensor contents.
"""

import os

import numpy as np
from contextlib import ExitStack

os.environ.setdefault("NEURON_RT_RESET_CORES", "1")

N = 100000
E = 600000
D = 128
NCORES = 8
ALPHA = 0.5

# gather source windows (int16 idx => each <= 32768 rows). Window 0 is
# deliberately small: the first dir-1 gathers wait on its xb prescale.
WBOUNDS = [0, 16384, 44256, 72128, 100000]
CALL = 1024          # max tokens per dma_gather call
SEG = 2048           # prescale segment rows
SEGG = SEG // 128


def _cfg_for(n_nodes):
    assert n_nodes == WBOUNDS[-1]
    nw = len(WBOUNDS) - 1
    nloc = n_nodes // NCORES
    return dict(N=n_nodes, NW=nw, NLOC=nloc,
                NBLK=(nloc + 127) // 128)


def _wrap_idx(arr):
    b = arr.shape[0]
    assert b % 16 == 0
    t = arr.reshape(b // 16, 16).T.copy()
    return np.tile(t, (8, 1)).astype(np.int16)


def _prep_host(x, edge_index, W_src, b_src, W_dst, b_dst, cfg):
    """Pure index reorganization on host -> shared plan + per-core inputs."""
    n, nw, nloc, nblk = cfg["N"], cfg["NW"], cfg["NLOC"], cfg["NBLK"]
    row = np.asarray(edge_index[0], dtype=np.int64)
    col = np.asarray(edge_index[1], dtype=np.int64)

    rp_row = np.zeros(n + 1, dtype=np.int64)
    rp_row[1:] = np.cumsum(np.bincount(row, minlength=n))
    rp_col = np.zeros(n + 1, dtype=np.int64)
    rp_col[1:] = np.cumsum(np.bincount(col, minlength=n))

    def rp_prescale(rp):
        cols = []
        for w in range(nw):
            base = WBOUNDS[w]
            rows_w = WBOUNDS[w + 1] - base
            nseg = rows_w // SEG
            for si in range(nseg):
                cols.append(base + si * SEG
                            + np.arange(128)[:, None] * SEGG
                            + np.arange(SEGG)[None, :])
            r0 = base + nseg * SEG
            rem = rows_w - nseg * SEG
            t1 = rem // 128
            if t1:
                cols.append(r0 + np.arange(t1)[None, :] * 128
                            + np.arange(128)[:, None])
            t2 = rem - t1 * 128
            if t2:
                c = r0 + t1 * 128 + np.arange(128)[:, None]
                cols.append(np.where(c < base + rows_w, c, n))
        idx = np.concatenate(cols, axis=1)
        idx = np.minimum(idx, n)
        lo = rp[idx].astype(np.int32)
        hi = rp[np.minimum(idx + 1, n)].astype(np.int32)
        return lo, hi

    rpa_lo, rpa_hi = rp_prescale(rp_row)
    rpb_lo, rpb_hi = rp_prescale(rp_col)

    def rp_local(rp, c):
        idx = (np.arange(nblk)[None, :] * 128 + np.arange(128)[:, None])
        valid = idx < nloc
        idx = np.minimum(c * nloc + idx, n)
        lo = rp[idx]
        hi = rp[np.minimum(idx + 1, n)]
        hi = np.where(valid, hi, lo)
        return lo.astype(np.int32), hi.astype(np.int32)

    def bucket(dest, src):
        """dest-sorted token streams. Returns (plan, g_list, dloc_list)."""
        core = dest // nloc
        pc = []
        cnt = np.zeros((NCORES, nw, nblk), np.int64)
        for c in range(NCORES):
            m = core == c
            d = (dest[m] - c * nloc).astype(np.int64)
            s = src[m].astype(np.int64)
            w = np.searchsorted(WBOUNDS, s, side="right") - 1
            b = d >> 7
            o = np.lexsort((d, b, w))
            d, s, w, b = (d[o], s[o] - np.asarray(WBOUNDS)[w[o]], w[o],
                          b[o])
            np.add.at(cnt[c], (w, b), 1)
            pc.append((d, s, w, b))
        size_wb = cnt.max(axis=0)                      # [nw, nblk] static
        starts = np.zeros((nw, nblk + 1), np.int64)
        starts[:, 1:] = np.cumsum(size_wb, axis=1)
        wtot = starts[:, -1]
        ntokw = ((wtot + 127) // 128) * 128            # window padded to x128
        win_tok0 = np.zeros(nw, np.int64)
        win_tok0[1:] = np.cumsum(ntokw)[:-1]
        total = int(ntokw.sum())

        # static chunk -> block matmul schedule
        windows = []
        for w in range(nw):
            nch = int(ntokw[w]) // 128
            mms = []                                   # (chunk, block)
            for ci in range(nch):
                lo_t, hi_t = ci * 128, ci * 128 + 128
                for b in range(nblk):
                    if size_wb[w, b] > 0 and starts[w, b] < hi_t \
                            and starts[w, b + 1] > lo_t:
                        mms.append((ci, b))
            calls = [(a, min(CALL, int(ntokw[w]) - a))
                     for a in range(0, int(ntokw[w]), CALL)]
            # psum segment bounds: first/last mm index per block
            seg_first, seg_last = {}, {}
            for j, (ci, b) in enumerate(mms):
                seg_first.setdefault(b, j)
                seg_last[b] = j
            windows.append(dict(tok0=int(win_tok0[w]), ntok=int(ntokw[w]),
                                calls=calls, mms=mms,
                                seg_first=seg_first, seg_last=seg_last))

        g_list, dl_list = [], []
        nmm = sum(len(wd["mms"]) for wd in windows)
        for c in range(NCORES):
            d, s, w, b = pc[c]
            key = w * nblk + b
            gs0 = np.r_[0, np.cumsum(np.bincount(key, minlength=nw * nblk))]
            rank = np.arange(len(d)) - gs0[key]
            pos = win_tok0[w] + starts[w, b] + rank
            g = np.zeros(total, np.int64)
            dl = -np.ones(total, np.int64)
            g[pos] = s
            dl[pos] = d
            # per-mm one-hot dest columns (local id within block or -1)
            cols = np.empty((nmm, 128), np.int16)
            j = 0
            for w2, wd in enumerate(windows):
                dlw = dl[wd["tok0"]:wd["tok0"] + wd["ntok"]].reshape(-1, 128)
                for (ci, b2) in wd["mms"]:
                    r = dlw[ci]
                    cols[j] = np.where((r >= b2 * 128) & (r < (b2 + 1) * 128),
                                       r - b2 * 128, -1).astype(np.int16)
                    j += 1
            g_list.append(_wrap_idx(g.astype(np.int16)))
            dl_list.append(np.ascontiguousarray(cols.T))   # [128, nmm]
        return dict(windows=windows, total=total, nmm=nmm), g_list, dl_list

    plan1, g1, dl1 = bucket(row, col)
    plan2, g2, dl2 = bucket(col, row)

    wsrcT = np.ascontiguousarray(np.asarray(W_src, np.float32).T)
    wdstT = np.ascontiguousarray(np.asarray(W_dst, np.float32).T)
    xf = np.ascontiguousarray(np.asarray(x, np.float32))
    iota = np.tile(np.arange(128, dtype=np.float32)[None, :], (128, 1))

    in_maps = []
    for c in range(NCORES):
        rp1_lo, rp1_hi = rp_local(rp_row, c)
        rp2_lo, rp2_hi = rp_local(rp_col, c)
        in_maps.append({
            "x": xf, "wsrcT": wsrcT, "wdstT": wdstT, "iota": iota,
            "bsrc": np.asarray(b_src, np.float32),
            "bdst": np.asarray(b_dst, np.float32),
            "g1": g1[c], "g2": g2[c],
            "dl1": dl1[c], "dl2": dl2[c],
            "rpa_lo": rpa_lo, "rpa_hi": rpa_hi,
            "rpb_lo": rpb_lo, "rpb_hi": rpb_hi,
            "rp1_lo": rp1_lo, "rp1_hi": rp1_hi,
            "rp2_lo": rp2_lo, "rp2_hi": rp2_hi,
        })
    return in_maps, plan1, plan2


def _build(cfg, plan1, plan2):
    import concourse.tile as tile
    from concourse import bacc, mybir

    dt = mybir.dt
    n, nw = cfg["N"], cfg["NW"]
    nloc, nblk = cfg["NLOC"], cfg["NBLK"]

    nc = bacc.Bacc("TRN2", target_bir_lowering=False, debug=False,
                   num_devices=NCORES)

    x = nc.dram_tensor("x", [n, D], dt.float32, kind="ExternalInput")
    wsrcT = nc.dram_tensor("wsrcT", [D, D], dt.float32, kind="ExternalInput")
    wdstT = nc.dram_tensor("wdstT", [D, D], dt.float32, kind="ExternalInput")
    iota = nc.dram_tensor("iota", [D, D], dt.float32, kind="ExternalInput")
    bsrc = nc.dram_tensor("bsrc", [D], dt.float32, kind="ExternalInput")
    bdst = nc.dram_tensor("bdst", [D], dt.float32, kind="ExternalInput")
    g1 = nc.dram_tensor("g1", [128, plan1["total"] // 16], dt.int16,
                        kind="ExternalInput")
    g2 = nc.dram_tensor("g2", [128, plan2["total"] // 16], dt.int16,
                        kind="ExternalInput")
    dl1 = nc.dram_tensor("dl1", [128, plan1["nmm"]], dt.int16,
                         kind="ExternalInput")
    dl2 = nc.dram_tensor("dl2", [128, plan2["nmm"]], dt.int16,
                         kind="ExternalInput")

    presched = []
    pcols = 0
    for w in range(nw):
        base = WBOUNDS[w]
        rows_w = WBOUNDS[w + 1] - base
        nseg = rows_w // SEG
        steps = []
        for si in range(nseg):
            steps.append(("seg", base + si * SEG, SEG, SEGG))
            pcols += SEGG
        r0 = base + nseg * SEG
        rem = rows_w - nseg * SEG
        t1 = rem // 128
        if t1:
            steps.append(("t1", r0, t1 * 128, t1))
            pcols += t1
        t2 = rem - t1 * 128
        if t2:
            steps.append(("t2", r0 + t1 * 128, t2, 1))
            pcols += 1
        presched.append(steps)

    rpa_lo = nc.dram_tensor("rpa_lo", [128, pcols], dt.int32, kind="ExternalInput")
    rpa_hi = nc.dram_tensor("rpa_hi", [128, pcols], dt.int32, kind="ExternalInput")
    rpb_lo = nc.dram_tensor("rpb_lo", [128, pcols], dt.int32, kind="ExternalInput")
    rpb_hi = nc.dram_tensor("rpb_hi", [128, pcols], dt.int32, kind="ExternalInput")
    rp1_lo = nc.dram_tensor("rp1_lo", [128, nblk], dt.int32, kind="ExternalInput")
    rp1_hi = nc.dram_tensor("rp1_hi", [128, nblk], dt.int32, kind="ExternalInput")
    rp2_lo = nc.dram_tensor("rp2_lo", [128, nblk], dt.int32, kind="ExternalInput")
    rp2_hi = nc.dram_tensor("rp2_hi", [128, nblk], dt.int32, kind="ExternalInput")
    out = nc.dram_tensor("out", [nloc, D], dt.float32, kind="ExternalOutput")

    xbw, xaw = [], []
    for w in range(nw):
        rows_w = WBOUNDS[w + 1] - WBOUNDS[w]
        xbw.append(nc.dram_tensor(f"xb{w}", [rows_w, D], dt.bfloat16))
        xaw.append(nc.dram_tensor(f"xa{w}", [rows_w, D], dt.bfloat16))

    with tile.TileContext(nc) as tc, ExitStack() as ctx:
        const = ctx.enter_context(tc.tile_pool(name="const", bufs=1))

        wsrcT_sb = const.tile([D, D], dt.float32, tag="wsrc")
        nc.sync.dma_start(wsrcT_sb[:], wsrcT.ap())
        wdstT_sb = const.tile([D, D], dt.float32, tag="wdst")
        nc.sync.dma_start(wdstT_sb[:], wdstT.ap())
        iota_sb = const.tile([D, D], dt.float32, tag="iota")
        nc.sync.dma_start(iota_sb[:], iota.ap())

        brow = const.tile([1, 2 * D], dt.float32, tag="brow")
        nc.sync.dma_start(brow[:, 0:D], bsrc.ap().unsqueeze(0))
        nc.sync.dma_start(brow[:, D:2 * D], bdst.ap().unsqueeze(0))
        bsum = const.tile([1, D], dt.float32, tag="bsum")
        nc.vector.tensor_scalar_mul(bsum[:], brow[:, 0:D], 1.0 - ALPHA)
        bs2 = const.tile([1, D], dt.float32, tag="bs2")
        nc.vector.tensor_scalar_mul(bs2[:], brow[:, D:2 * D], ALPHA)
        nc.vector.tensor_add(bsum[:], bsum[:], bs2[:])
        bias_bc = const.tile([D, D], dt.float32, tag="biasbc")
        nc.gpsimd.partition_broadcast(bias_bc[:], bsum[:])

        g1_sb = const.tile([128, plan1["total"] // 16], dt.int16, tag="g1")
        nc.sync.dma_start(g1_sb[:], g1.ap())
        g2_sb = const.tile([128, plan2["total"] // 16], dt.int16, tag="g2")
        nc.sync.dma_start(g2_sb[:], g2.ap())

        dlf1 = const.tile([128, plan1["nmm"]], dt.float32, tag="dlf1")
        dlf2 = const.tile([128, plan2["nmm"]], dt.float32, tag="dlf2")

        def invsqrt_chain(pool, lo_ap, hi_ap, cols, tag, scale=None,
                          res_pool=None):
            res_pool = res_pool or pool
            lo_t = pool.tile([128, cols], dt.int32, tag=tag + "lo")
            nc.sync.dma_start(lo_t[:], lo_ap)
            hi_t = pool.tile([128, cols], dt.int32, tag=tag + "hi")
            nc.sync.dma_start(hi_t[:], hi_ap)
            deg_i = pool.tile([128, cols], dt.int32, tag=tag + "di")
            nc.vector.tensor_sub(deg_i[:], hi_t[:], lo_t[:])
            deg_f = pool.tile([128, cols], dt.float32, tag=tag + "df")
            nc.vector.tensor_copy(deg_f[:], deg_i[:])
            mask = pool.tile([128, cols], dt.float32, tag=tag + "mk")
            mul = scale if scale is not None else 1.0
            nc.vector.tensor_scalar(mask[:], deg_f[:], 1.0, mul,
                                    mybir.AluOpType.min, mybir.AluOpType.mult)
            dmax = pool.tile([128, cols], dt.float32, tag=tag + "dm")
            nc.vector.tensor_scalar_max(dmax[:], deg_f[:], 1.0)
            rec = pool.tile([128, cols], dt.float32, tag=tag + "rc")
            nc.vector.reciprocal(rec[:], dmax[:])
            sq = pool.tile([128, cols], dt.float32, tag=tag + "sq")
            nc.scalar.sqrt(sq[:], rec[:])
            res = res_pool.tile([128, cols], dt.float32, tag=tag + "rs")
            nc.vector.tensor_mul(res[:], sq[:], mask[:])
            return res

        with tc.tile_pool(name="chainscratch", bufs=1) as csp:
            a_vec = invsqrt_chain(csp, rp1_lo.ap(), rp1_hi.ap(), nblk, "av",
                                  scale=1.0 - ALPHA, res_pool=const)
            b_vec = invsqrt_chain(csp, rp2_lo.ap(), rp2_hi.ap(), nblk, "bv",
                                  scale=ALPHA, res_pool=const)
            b_full = invsqrt_chain(csp, rpb_lo.ap(), rpb_hi.ap(), pcols,
                                   "bf", res_pool=const)
            a_full = invsqrt_chain(csp, rpa_lo.ap(), rpa_hi.ap(), pcols,
                                   "af", res_pool=const)
            di1 = csp.tile([128, plan1["nmm"]], dt.int16, tag="di1")
            nc.sync.dma_start(di1[:], dl1.ap())
            nc.vector.tensor_copy(dlf1[:], di1[:])
            di2 = csp.tile([128, plan2["nmm"]], dt.int16, tag="di2")
            nc.sync.dma_start(di2[:], dl2.ap())
            nc.vector.tensor_copy(dlf2[:], di2[:])

        # SBUF accumulators [feat x dest], one per direction
        agg1_sb = const.tile([128, nblk * 128], dt.float32, tag="agg1")
        agg2_sb = const.tile([128, nblk * 128], dt.float32, tag="agg2")

        def zero_aggs():
            for agg in (agg1_sb, agg2_sb):
                off = 0
                while off < nblk * 128:
                    csz = min(4096, nblk * 128 - off)
                    nc.vector.memset(agg[:, off:off + csz], 0.0)
                    off += csz

        gpool = ctx.enter_context(tc.tile_pool(name="gat", bufs=6))
        spool = ctx.enter_context(tc.tile_pool(name="sb", bufs=6))
        epsum = ctx.enter_context(tc.tile_pool(name="eps", bufs=4,
                                               space="PSUM"))

        def prescale_window(pp, w, which, ccur):
            dest, sv = (xbw[w], b_full) if which == "b" else (xaw[w], a_full)
            for kind_, r0, nrows, ncols in presched[w]:
                base = WBOUNDS[w]
                cs = slice(ccur, ccur + ncols)
                ccur += ncols
                if kind_ == "t2":
                    xt = pp.tile([nrows, D], dt.float32, tag="pxt2")
                    nc.sync.dma_start(xt[:], x.ap()[r0:r0 + nrows, :])
                    ot = pp.tile([nrows, D], dt.bfloat16, tag="pot2")
                    nc.scalar.mul(ot[:], xt[:], sv[0:nrows, cs])
                    nc.sync.dma_start(
                        dest.ap()[r0 - base:r0 - base + nrows, :], ot[:])
                    continue
                wrap = "(p g) d -> p g d" if kind_ == "seg" \
                    else "(g p) d -> p g d"
                xs = x.ap()[r0:r0 + nrows, :].rearrange(wrap, p=128)
                xt = pp.tile([128, ncols, D], dt.float32, tag="pxt")
                nc.sync.dma_start(xt[:], xs)
                ex = pp.tile([128, ncols, D], dt.bfloat16, tag="pex")
                nc.vector.tensor_tensor(
                    ex[:], sv[:, cs].unsqueeze(2).to_broadcast(
                        [128, ncols, D]),
                    xt[:], mybir.AluOpType.mult)
                dv = dest.ap()[r0 - base:r0 - base + nrows, :] \
                    .rearrange(wrap, p=128)
                nc.sync.dma_start(dv, ex[:])
            return ccur

        def edge_window(plan, w, g_sb, dlf, srcw, agg_sb, mm0, dtag,
                        final_cb=None):
            wd = plan["windows"][w]
            xs = srcw[w].ap()
            tiles = {}
            for (a, ln) in wd["calls"]:
                xt = gpool.tile([128, CALL // 128, D], dt.bfloat16,
                                tag="xt" + dtag)
                o = wd["tok0"] + a
                gi = g_sb[:, o // 16:(o + ln) // 16]
                nc.gpsimd.dma_gather(xt[:, 0:ln // 128, :], xs, gi, ln, ln, D)
                for g in range(ln // 128):
                    tiles[a // 128 + g] = (xt, g)
            mms = wd["mms"]
            # batched one-hot builds (8 consecutive mm columns per op)
            sb_tiles = []
            for j0 in range(0, len(mms), 8):
                jn = min(8, len(mms) - j0)
                st = spool.tile([128, 8, D], dt.bfloat16, tag="st")
                nc.vector.tensor_tensor(
                    st[:, 0:jn, :],
                    dlf[:, mm0 + j0:mm0 + j0 + jn].unsqueeze(2)
                    .to_broadcast([128, jn, D]),
                    iota_sb[:].unsqueeze(1).to_broadcast([128, jn, D]),
                    mybir.AluOpType.is_equal)
                sb_tiles.append(st)
            active = {}
            for j, (ci, b) in enumerate(mms):
                xt, g = tiles[ci]
                st = sb_tiles[j // 8]
                if b not in active:
                    active[b] = epsum.tile([128, D], dt.float32, tag="ep",
                                           name="ep")
                ps = active[b]
                nc.tensor.matmul(ps[:], lhsT=xt[:, g, :], rhs=st[:, j % 8, :],
                                 start=(j == wd["seg_first"][b]),
                                 stop=(j == wd["seg_last"][b]))
                if j == wd["seg_last"][b]:
                    nc.vector.tensor_add(
                        agg_sb[:, b * 128:(b + 1) * 128],
                        agg_sb[:, b * 128:(b + 1) * 128], ps[:])
                    del active[b]
                    if final_cb is not None:
                        final_cb(b)
            return mm0 + len(mms)

        # final per-block output, emitted as soon as a block's accumulators
        # are complete (interleaved into the last window's dir-2 stream)
        fp = ctx.enter_context(tc.tile_pool(name="fin", bufs=3))
        fps = ctx.enter_context(tc.tile_pool(name="fps", bufs=2,
                                             space="PSUM"))

        def final_block(k):
            ks = slice(k * 128, (k + 1) * 128)
            p1 = fps.tile([128, D], dt.float32, tag="p1", name="p1")
            nc.tensor.matmul(p1[:], lhsT=agg1_sb[:, ks], rhs=wsrcT_sb[:],
                             start=True, stop=True)
            p2 = fps.tile([128, D], dt.float32, tag="p2", name="p2")
            nc.tensor.matmul(p2[:], lhsT=agg2_sb[:, ks], rhs=wdstT_sb[:],
                             start=True, stop=True)
            o1 = fp.tile([128, D], dt.float32, tag="o1", name="o1")
            nc.scalar.mul(o1[:], p1[:], a_vec[:, k:k + 1])
            o2 = fp.tile([128, D], dt.float32, tag="o2", name="o2")
            nc.scalar.mul(o2[:], p2[:], b_vec[:, k:k + 1])
            fin = fp.tile([128, D], dt.float32, tag="fin", name="fin")
            nc.vector.tensor_add(fin[:], o1[:], o2[:])
            nc.vector.tensor_add(fin[:], fin[:], bias_bc[:])
            rows = min(128, nloc - k * 128)
            nc.sync.dma_start(out.ap()[k * 128:k * 128 + rows, :],
                              fin[0:rows, :])

        emitted = set()

        def final_cb(b):
            if b not in emitted:
                emitted.add(b)
                final_block(b)

        # interleave: xb prescale feeds dir-1 gathers, xa feeds dir-2;
        # window w+1 prescale is emitted before window w's edge stream
        with tc.tile_pool(name="prescale", bufs=2) as pp:
            ccb, cca, mm1, mm2 = [0], [0], [0], [0]

            def pre_b(w):
                ccb[0] = prescale_window(pp, w, "b", ccb[0])

            def pre_a(w):
                cca[0] = prescale_window(pp, w, "a", cca[0])

            def e1(w, cb=None):
                mm1[0] = edge_window(plan1, w, g1_sb, dlf1, xbw, agg1_sb,
                                     mm1[0], "1", cb)

            def e2(w, cb=None):
                mm2[0] = edge_window(plan2, w, g2_sb, dlf2, xaw, agg2_sb,
                                     mm2[0], "2", cb)

            pre_b(0)
            zero_aggs()
            if nw > 1:
                pre_b(1)
            pre_a(0)
            e1(0)
            if nw > 1:
                pre_a(1)
            e2(0)
            for w in range(1, nw - 1):
                pre_b(w + 1)
                e1(w)
                pre_a(w + 1)
                e2(w)
            if nw > 1:
                e1(nw - 1)
                e2(nw - 1, final_cb)
        for k in range(nblk):
            if k not in emitted:
                emitted.add(k)
                final_block(k)

    nc.compile()
    return nc


def _install_ntff_shim():
    """This image's antenv lacks axon_hooks; inject it so trace=True works."""
    import sys
    import types
    try:
        from antenv import axon_hooks  # noqa: F401
        return
    except ImportError:
        pass
    try:
        import antenv
        from trn_agent_boot.trn_boot import _ntff_profile_via_ctypes
        mod = types.ModuleType("antenv.axon_hooks")
        holder = [None]
        mod.set_axon_ntff_profile_hook = lambda h: holder.__setitem__(0, h)
        mod.get_axon_ntff_profile_hook = lambda: holder[0]
        sys.modules["antenv.axon_hooks"] = mod
        antenv.axon_hooks = mod
        mod.set_axon_ntff_profile_hook(
            _ntff_profile_via_ctypes("/opt/axon/libaxon_pjrt.so"))
    except Exception as e:  # profiling is best-effort
        print("ntff shim failed:", e)


def _run(nc, in_maps, trace=False):
    from concourse.bass_utils import run_bass_kernel_spmd
    kw = {}
    if trace:
        _install_ntff_shim()
        kw = dict(trace=True, trace_cores=list(range(NCORES)))
    return run_bass_kernel_spmd(nc, in_maps, list(range(NCORES)), **kw)


def kernel(x, edge_index, W_src, b_src, W_dst, b_dst, _trace=False,
           _return_result=False):
    cfg = _cfg_for(x.shape[0])
    in_maps, plan1, plan2 = _prep_host(x, edge_index, W_src, b_src, W_dst,
                                       b_dst, cfg)
    nc = _build(cfg, plan1, plan2)
    res = _run(nc, in_maps, trace=_trace)
    out = np.concatenate([res.results[c]["out"] for c in range(NCORES)],
                         axis=0)
    if _return_result:
        return out, res
    return out

